# revision 10
# baseline (speedup 1.0000x reference)
"""Multi-head attention (B=2, T=2048, C=1024, H=16) on 8 trn2 cores.

Sharding: core c -> batch b = c//4, head-group g = c%4 (4 heads, proj cols
[g*256, (g+1)*256)).  Host pre-transposes per-batch inputs to feature-major
[C, T] so every device matmul has its contraction dim on SBUF partitions.
Each core computes a partial output  O_g @ Wo[g-rows]  [2048, 1024]; the
host sums the 4 partials per batch and adds bo.
"""

import ml_dtypes
import numpy as np


import concourse.bass as bass
import concourse.tile as tile
from concourse import bacc, mybir
from concourse.bass_utils import run_bass_kernel_spmd

B, T, C, H, D = 2, 2048, 1024, 16, 64
N_CORES = 8
GROUPS = 4          # head-groups (cores per batch)
HG = H // GROUPS    # heads per core = 4
CG = HG * D         # proj cols per core = 256
KT = C // 128       # contraction k-tiles = 8
SCALE = D ** -0.5   # 1/8

F32 = mybir.dt.float32
F32R = mybir.dt.float32r
BF16 = mybir.dt.bfloat16
AF = mybir.ActivationFunctionType


def build_mha_program():
    """Build the SPMD Bass program (identical on all 8 cores)."""
    nc = bacc.Bacc("TRN2", target_bir_lowering=False, debug=False,
                   num_devices=N_CORES)

    xqT = nc.dram_tensor("xqT", (C, T), BF16, kind="ExternalInput").ap()
    xkT = nc.dram_tensor("xkT", (C, T), BF16, kind="ExternalInput").ap()
    xvT = nc.dram_tensor("xvT", (C, T), BF16, kind="ExternalInput").ap()
    wq = nc.dram_tensor("wq", (C, CG), BF16, kind="ExternalInput").ap()
    wk = nc.dram_tensor("wk", (C, CG), BF16, kind="ExternalInput").ap()
    wv = nc.dram_tensor("wv", (C, CG), BF16, kind="ExternalInput").ap()
    bq = nc.dram_tensor("bq", (CG,), F32, kind="ExternalInput").ap()
    bk = nc.dram_tensor("bk", (CG,), F32, kind="ExternalInput").ap()
    bv = nc.dram_tensor("bv", (CG,), F32, kind="ExternalInput").ap()
    wo = nc.dram_tensor("wo", (CG, C), BF16, kind="ExternalInput").ap()
    yp = nc.dram_tensor("yp", (T, C), F32, kind="ExternalOutput").ap()

    with tile.TileContext(nc) as tc:
        _emit(tc, xqT, xkT, xvT, wq, wk, wv, bq, bk, bv, wo, yp)
    nc.compile()
    return nc


def _emit(tc, xqT, xkT, xvT, wq, wk, wv, bq, bk, bv, wo, yp):
    nc = tc.nc
    MT = CG // 128            # stationary tiles per projection = 2
    MC = 256                  # stage-A/B moving-chunk width
    NMC = T // MC             # 8 chunks
    TT = T // 128             # 16 t-tiles
    QC = 512                  # q-chunk width in attention
    NQC = T // QC             # 4 q-chunks
    VS = D + 1                # 65: V cols + ones col per head

    from contextlib import ExitStack
    with ExitStack() as ctx:
        consts = ctx.enter_context(tc.tile_pool(name="consts", bufs=1))
        xs_pool = ctx.enter_context(tc.tile_pool(name="xs", bufs=2))
        big = ctx.enter_context(tc.tile_pool(name="big", bufs=1))
        e_pool = ctx.enter_context(tc.tile_pool(name="e", bufs=3))
        ev_pool = ctx.enter_context(tc.tile_pool(name="ev", bufs=2))
        nrm_pool = ctx.enter_context(tc.tile_pool(name="nrm", bufs=2))
        pp = ctx.enter_context(tc.tile_pool(name="pp", bufs=2, space="PSUM"))
        sa_ps = ctx.enter_context(tc.tile_pool(name="saps", bufs=1, space="PSUM"))
        sb_ps = ctx.enter_context(tc.tile_pool(name="sbps", bufs=1, space="PSUM"))

        # ---- constants -------------------------------------------------
        wq_s = consts.tile([128, KT, CG], BF16, tag="wq")
        wk_s = consts.tile([128, KT, CG], BF16, tag="wk")
        wv_s = consts.tile([128, KT, CG], BF16, tag="wv")
        for w_s, w_d in ((wq_s, wq), (wk_s, wk), (wv_s, wv)):
            nc.sync.dma_start(out=w_s, in_=w_d.rearrange("(kt p) c -> p kt c", p=128))
        # per-head rows at partition base 0: [64, h, c]
        wo_s = consts.tile([64, HG, C], BF16, tag="wo")
        nc.sync.dma_start(out=wo_s, in_=wo.rearrange("(h p) c -> p h c", p=64))
        ones_f = consts.tile([128, D], F32, tag="onesf")
        nc.vector.memset(ones_f[D:D + 1, :], 1.0)
        ones_t = consts.tile([128, D], F32R, tag="ones")
        nc.vector.tensor_copy(ones_t[D:D + 1, :], ones_f[D:D + 1, :])
        bq_s = consts.tile([128, MT, 1], F32, tag="bq")
        bk_s = consts.tile([128, MT, 1], F32, tag="bk")
        nc.sync.dma_start(
            out=bq_s, in_=bq.rearrange("(mt p) -> p mt", p=128).unsqueeze(2))
        nc.sync.dma_start(
            out=bk_s, in_=bk.rearrange("(mt p) -> p mt", p=128).unsqueeze(2))
        bv_bc = consts.tile([128, CG], F32, tag="bv")
        nc.sync.dma_start(
            out=bv_bc,
            in_=bass.AP(tensor=bv.tensor, offset=bv.offset,
                        ap=[[0, 128]] + list(bv.ap)),
        )

        # ---- persistent activations -----------------------------------
        qT_s = big.tile([128, MT, T], BF16, tag="qT")   # [d, (mt), t]
        kT_s = big.tile([128, MT, T], BF16, tag="kT")
        v_s = big.tile([128, TT, HG * VS], BF16, tag="v")  # ones col at 64/head
        oT_s = big.tile([64, HG, T], BF16, tag="oT")       # per-head, base 0
        nc.vector.memset(
            v_s.rearrange("p t (h c) -> p t h c", h=HG)[:, :, :, D:VS], 1.0)

        # ---- stage A: Q^T, K^T = (W.T @ X^T) + b ----------------------
        for mc in range(NMC):
            cols = bass.ts(mc, MC)
            xq_t = xs_pool.tile([128, KT, MC], BF16, tag="xs")
            nc.sync.dma_start(out=xq_t,
                              in_=xqT[:, cols].rearrange("(kt p) m -> p kt m", p=128))
            xk_t = xs_pool.tile([128, KT, MC], BF16, tag="xs")
            nc.sync.dma_start(out=xk_t,
                              in_=xkT[:, cols].rearrange("(kt p) m -> p kt m", p=128))
            for x_t, w_s, b_s, dst in ((xq_t, wq_s, bq_s, qT_s),
                                       (xk_t, wk_s, bk_s, kT_s)):
                for mt in range(MT):
                    ps = pp.tile([128, 512], F32, tag="pp")
                    for kt in range(KT):
                        nc.tensor.matmul(
                            ps[:, :MC],
                            w_s[:, kt, bass.ts(mt, 128)],
                            x_t[:, kt, :],
                            start=(kt == 0), stop=(kt == KT - 1))
                    nc.vector.tensor_scalar_add(
                        dst[:, mt, cols], ps[:, :MC], b_s[:, mt, :])

        # ---- stage B: V natural [t, 256] + ones cols ------------------
        v4 = v_s.rearrange("p t (h c) -> p t h c", h=HG)
        for mc in range(NMC):
            cols = bass.ts(mc, MC)
            xv_t = xs_pool.tile([128, KT, MC], BF16, tag="xs")
            nc.sync.dma_start(out=xv_t,
                              in_=xvT[:, cols].rearrange("(kt p) m -> p kt m", p=128))
            for sub in range(MC // 128):
                tt = mc * (MC // 128) + sub
                ps = pp.tile([128, 512], F32, tag="pp")
                for kt in range(KT):
                    nc.tensor.matmul(
                        ps[:, :CG],
                        xv_t[:, kt, bass.ts(sub, 128)],
                        wv_s[:, kt, :],
                        start=(kt == 0), stop=(kt == KT - 1))
                nc.vector.tensor_add(
                    v4[:, tt, :, 0:D],
                    ps[:, :CG].rearrange("p (h c) -> p h c", h=HG),
                    bv_bc.rearrange("p (h c) -> p h c", h=HG))

        # ---- stage C: attention per (head, q-chunk) -------------------
        # exp-group pattern over the 16 key tiles: 4,2,4,2,4 (psum banks)
        GROUPS_TK = ((sa_ps, 4), (sb_ps, 2), (sa_ps, 4), (sb_ps, 2), (sa_ps, 4))
        for h in range(HG):
            mt = h // 2
            pb = (h % 2) * D          # partition base of this head's d rows
            vcols = bass.ds(h * VS, VS)
            for qc in range(NQC):
                qcols = bass.ts(qc, QC)
                rhs_q = qT_s[pb:pb + D, mt, qcols]
                po = pp.tile([128, 512], F32, tag="pp")
                tk0 = 0
                for pool, glen in GROUPS_TK:
                    ps = pool.tile([128, glen, 512], F32,
                                   tag="sa" if glen == 4 else "sb")
                    for j in range(glen):
                        tk = tk0 + j
                        nc.tensor.matmul(
                            ps[:, j, :],
                            kT_s[pb:pb + D, mt, bass.ts(tk, 128)],
                            rhs_q,
                            start=True, stop=True)
                    e_t = e_pool.tile([128, glen, 512], BF16, tag="e")
                    nc.scalar.activation(e_t, ps, AF.Exp, scale=SCALE)
                    for j in range(glen):
                        tk = tk0 + j
                        nc.tensor.matmul(
                            po[0:VS, :],
                            v_s[:, tk, vcols],
                            e_t[:, j, :],
                            start=(tk == 0), stop=(tk == TT - 1))
                    tk0 += glen
                # normalize: row D of po holds sum(exp).  Broadcast it across
                # partitions with a K=1 matmul against a ones row, then
                # reciprocal + multiply (all partition-base-0 aligned).
                den = nrm_pool.tile([128, 512], F32R, tag="den")
                nc.vector.tensor_copy(den[D:D + 1, :], po[D:D + 1, :])
                rb = pp.tile([128, 512], F32, tag="pp")
                nc.tensor.matmul(rb[0:D, :],
                                 ones_t[D:D + 1, :],
                                 den[D:D + 1, :],
                                 start=True, stop=True)
                rec = nrm_pool.tile([128, 512], F32, tag="rec")
                nc.vector.reciprocal(rec[0:D, :], rb[0:D, :])
                nc.vector.tensor_mul(
                    oT_s[:, h, qcols], po[0:D, :], rec[0:D, :])

        # ---- stage D: Y partial = O @ Wo_g ----------------------------
        for tt in range(TT):
            trows = bass.ts(tt, 128)
            for cc in range(C // 512):
                ps = pp.tile([128, 512], F32, tag="pp")
                for h in range(HG):
                    nc.tensor.matmul(
                        ps,
                        oT_s[:, h, trows],
                        wo_s[:, h, bass.ts(cc, 512)],
                        start=(h == 0), stop=(h == HG - 1))
                ev = ev_pool.tile([128, 512], F32, tag="ev")
                nc.vector.tensor_copy(ev, ps)
                nc.sync.dma_start(out=yp[trows, bass.ts(cc, 512)], in_=ev)


_NC_CACHE = None


def _get_program():
    global _NC_CACHE
    if _NC_CACHE is None:
        _NC_CACHE = build_mha_program()
    return _NC_CACHE


def make_in_maps(query, key, value, Wq, bq, Wk, bk, Wv, bv, Wo):
    q = np.asarray(query, np.float32).reshape(B, T, C)
    k = np.asarray(key, np.float32).reshape(B, T, C)
    v = np.asarray(value, np.float32).reshape(B, T, C)
    xT = {n: [np.ascontiguousarray(a[b].T).astype(ml_dtypes.bfloat16)
              for b in range(B)]
          for n, a in (("q", q), ("k", k), ("v", v))}
    in_maps = []
    for c in range(N_CORES):
        b, g = divmod(c, GROUPS)
        sl = slice(g * CG, (g + 1) * CG)
        in_maps.append({
            "xqT": xT["q"][b], "xkT": xT["k"][b], "xvT": xT["v"][b],
            "wq": np.ascontiguousarray(np.asarray(Wq, np.float32)[:, sl]).astype(ml_dtypes.bfloat16),
            "wk": np.ascontiguousarray(np.asarray(Wk, np.float32)[:, sl]).astype(ml_dtypes.bfloat16),
            "wv": np.ascontiguousarray(np.asarray(Wv, np.float32)[:, sl]).astype(ml_dtypes.bfloat16),
            "bq": np.ascontiguousarray(np.asarray(bq, np.float32)[sl]),
            "bk": np.ascontiguousarray(np.asarray(bk, np.float32)[sl]),
            "bv": np.ascontiguousarray(np.asarray(bv, np.float32)[sl]),
            "wo": np.ascontiguousarray(np.asarray(Wo, np.float32)[sl, :]).astype(ml_dtypes.bfloat16),
        })
    return in_maps


def assemble_output(results, bo):
    y = np.zeros((B, T, C), np.float32)
    for c, res in enumerate(results):
        y[c // GROUPS] += res["yp"]
    y += np.asarray(bo, np.float32)
    return y


def kernel(query, key, value, Wq, bq, Wk, bk, Wv, bv, Wo, bo):
    nc = _get_program()
    in_maps = make_in_maps(query, key, value, Wq, bq, Wk, bk, Wv, bv, Wo)
    res = run_bass_kernel_spmd(nc, in_maps, list(range(N_CORES)))
    return assemble_output(res.results, bo)


# revision 35
# speedup vs baseline: 2.3322x; 2.3322x over previous
"""Multi-head attention (B=2, T=2048, C=1024, H=16) on 8 trn2 cores.

Sharding: core c -> batch b = c//4, head-group g = c%4 (4 heads, proj cols
[g*256, (g+1)*256)).  Host pre-transposes per-batch inputs to feature-major
[C, T] so every device matmul has its contraction dim on SBUF partitions.
Each core computes a partial output  O_g @ Wo[g-rows]  [2048, 1024]; the
host sums the 4 partials per batch and adds bo.
"""

import ml_dtypes
import numpy as np


import concourse.bass as bass
import concourse.tile as tile
from concourse import bacc, mybir
from concourse.bass_utils import run_bass_kernel_spmd

B, T, C, H, D = 2, 2048, 1024, 16, 64
N_CORES = 8
GROUPS = 4          # head-groups (cores per batch)
HG = H // GROUPS    # heads per core = 4
CG = HG * D         # proj cols per core = 256
KT = C // 128       # contraction k-tiles = 8
SCALE = D ** -0.5   # 1/8

F32 = mybir.dt.float32
F32R = mybir.dt.float32r
BF16 = mybir.dt.bfloat16
AF = mybir.ActivationFunctionType



def build_mha_program():
    """Build the SPMD Bass program (identical on all 8 cores)."""
    nc = bacc.Bacc("TRN2", target_bir_lowering=False, debug=False,
                   num_devices=N_CORES)

    xqT = nc.dram_tensor("xqT", (C, T), BF16, kind="ExternalInput").ap()
    xkT = nc.dram_tensor("xkT", (C, T), BF16, kind="ExternalInput").ap()
    xvT = nc.dram_tensor("xvT", (C, T), BF16, kind="ExternalInput").ap()
    wq = nc.dram_tensor("wq", (C, CG), BF16, kind="ExternalInput").ap()
    wk = nc.dram_tensor("wk", (C, CG), BF16, kind="ExternalInput").ap()
    wv = nc.dram_tensor("wv", (C, CG), BF16, kind="ExternalInput").ap()
    bq = nc.dram_tensor("bq", (CG,), F32, kind="ExternalInput").ap()
    bk = nc.dram_tensor("bk", (CG,), F32, kind="ExternalInput").ap()
    bv = nc.dram_tensor("bv", (CG,), F32, kind="ExternalInput").ap()
    wo = nc.dram_tensor("wo", (CG, C), BF16, kind="ExternalInput").ap()
    yp = nc.dram_tensor("yp", (T, C), F32, kind="ExternalOutput").ap()

    with tile.TileContext(nc) as tc:
        _emit(tc, xqT, xkT, xvT, wq, wk, wv, bq, bk, bv, wo, yp)
    nc.compile()
    return nc


def _emit(tc, xqT, xkT, xvT, wq, wk, wv, bq, bk, bv, wo, yp):
    nc = tc.nc
    MT = CG // 128            # stationary tiles per projection = 2
    MC = 512                  # chunk width (tokens) everywhere
    NMC = T // MC             # 4 chunks
    TT = T // 128             # 16 t-tiles
    QC = 512                  # q-chunk width in attention
    NQC = T // QC             # 4 q-chunks
    VS = D + 1                # 65: V cols + ones col per head

    from contextlib import ExitStack
    with ExitStack() as ctx:
        consts = ctx.enter_context(tc.tile_pool(name="consts", bufs=1))
        xs_pool = ctx.enter_context(tc.tile_pool(name="xs", bufs=8))
        big = ctx.enter_context(tc.tile_pool(name="big", bufs=1))
        e_pool = ctx.enter_context(tc.tile_pool(name="e", bufs=6))
        ev_pool = ctx.enter_context(tc.tile_pool(name="ev", bufs=3))
        nrm_pool = ctx.enter_context(tc.tile_pool(name="nrm", bufs=4))
        pp = ctx.enter_context(tc.tile_pool(name="pp", bufs=2, space="PSUM"))
        pv_ps = ctx.enter_context(tc.tile_pool(name="pvps", bufs=2, space="PSUM"))
        sa_ps = ctx.enter_context(tc.tile_pool(name="saps", bufs=1, space="PSUM"))
        sb_ps = ctx.enter_context(tc.tile_pool(name="sbps", bufs=1, space="PSUM"))

        # Per-chunk persistent activations: fine-grained tiles so stages
        # pipeline at chunk granularity instead of a hard phase boundary.
        qTc = [big.tile([128, MT, MC], BF16, name=f"qTc{i}", tag=f"qTc{i}")
               for i in range(NMC)]
        kTc = [big.tile([128, MT, MC], BF16, name=f"kTc{i}", tag=f"kTc{i}")
               for i in range(NMC)]
        vc = [big.tile([128, MC // 128, HG * VS], BF16, name=f"vc{i}",
                       tag=f"vc{i}") for i in range(NMC)]
        oc = [big.tile([64, HG, QC], BF16, name=f"oc{i}", tag=f"oc{i}")
              for i in range(NQC)]

        wq_s = consts.tile([128, KT, CG], BF16, tag="wq")
        wk_s = consts.tile([128, KT, CG], BF16, tag="wk")
        wv_s = consts.tile([128, KT, CG], BF16, tag="wv")
        wo_s = consts.tile([64, HG, C], BF16, tag="wo")
        bq_s = consts.tile([128, MT, 1], F32, tag="bq")
        bk_s = consts.tile([128, MT, 1], F32, tag="bk")
        bv_bc = consts.tile([128, CG], F32, tag="bv")
        ones_f = consts.tile([128, D], F32, tag="onesf")
        ones_t = consts.tile([128, D], F32R, tag="ones")

        def load_x(src, name):
            x_t = xs_pool.tile([128, KT, MC], BF16, tag="xs", name=name)
            nc.sync.dma_start(out=x_t,
                              in_=src.rearrange("(kt p) m -> p kt m", p=128))
            return x_t

        def emit_a(mc):
            cols = bass.ts(mc, MC)
            xq_t = load_x(xqT[:, cols], f"xq{mc}")
            xk_t = load_x(xkT[:, cols], f"xk{mc}")
            for x_t, w_s, b_s, dstl in ((xq_t, wq_s, bq_s, qTc),
                                        (xk_t, wk_s, bk_s, kTc)):
                for mt in range(MT):
                    ps = pp.tile([128, 512], F32, tag="pp")
                    for kt in range(KT):
                        nc.tensor.matmul(
                            ps[:, :MC],
                            w_s[:, kt, bass.ts(mt, 128)],
                            x_t[:, kt, :],
                            start=(kt == 0), stop=(kt == KT - 1))
                    nc.vector.tensor_scalar_add(
                        dstl[mc][:, mt, :], ps[:, :MC], b_s[:, mt, :])

        def emit_b(mc):
            cols = bass.ts(mc, MC)
            v4 = vc[mc].rearrange("p t (h c) -> p t h c", h=HG)
            nc.vector.memset(v4[:, :, :, D:VS], 1.0)
            xv_t = load_x(xvT[:, cols], f"xv{mc}")
            for sub in range(MC // 128):
                ps = pp.tile([128, 512], F32, tag="pp")
                for kt in range(KT):
                    nc.tensor.matmul(
                        ps[:, :CG],
                        xv_t[:, kt, bass.ts(sub, 128)],
                        wv_s[:, kt, :],
                        start=(kt == 0), stop=(kt == KT - 1))
                nc.vector.tensor_add(
                    v4[:, sub, :, 0:D],
                    ps[:, :CG].rearrange("p (h c) -> p h c", h=HG),
                    bv_bc.rearrange("p (h c) -> p h c", h=HG))

        def emit_d_unit(qc, tl, cc):
            trows_out = bass.ts(qc * (QC // 128) + tl, 128)
            ps = pp.tile([128, 512], F32, tag="pp")
            for h in range(HG):
                nc.tensor.matmul(
                    ps,
                    oc[qc][:, h, bass.ts(tl, 128)],
                    wo_s[:, h, bass.ts(cc, 512)],
                    start=(h == 0), stop=(h == HG - 1))
            ev = ev_pool.tile([128, 512], F32, tag="ev")
            nc.vector.tensor_copy(ev, ps)
            nc.sync.dma_start(out=yp[trows_out, bass.ts(cc, 512)], in_=ev)

        def c_iter(qc, hp):
            """Attention for head pair hp on q-chunk qc.  Yields after each
            4-tk block so emission can be interleaved with stage A/B."""
            po = [pv_ps.tile([128, 512], F32, tag="pv",
                             name=f"po{qc}_{hp}_{i}") for i in range(2)]
            def emit_pv(e_prev, tkp):
                for h01 in range(2):
                    nc.tensor.matmul(
                        po[h01][0:VS, :],
                        vc[tkp // 4][:, tkp % 4,
                                     bass.ds((2 * hp + h01) * VS, VS)],
                        e_prev[:, h01, :],
                        start=(tkp == 0), stop=(tkp == TT - 1))

            pending = None    # issue PV one tk late so its exp-wait is
            for tk in range(TT):   # already satisfied at the queue head
                pool = sa_ps if tk % 2 == 0 else sb_ps
                ps = pool.tile([128, 2, 512], F32,
                               tag="sa" if tk % 2 == 0 else "sb")
                for h01 in range(2):
                    pb = h01 * D
                    nc.tensor.matmul(
                        ps[:, h01, :],
                        kTc[tk // 4][pb:pb + D, hp, bass.ts(tk % 4, 128)],
                        qTc[qc][pb:pb + D, hp, :],
                        start=True, stop=True)
                e_t = e_pool.tile([128, 2, 512], BF16, tag="e")
                nc.scalar.activation(e_t, ps, AF.Exp, scale=SCALE)
                if pending is not None:
                    emit_pv(*pending)
                pending = (e_t, tk)
                if tk % 4 == 3:
                    yield
            emit_pv(*pending)
            yield
            den = nrm_pool.tile([128, 2, 512], F32R, tag="den")
            for h01 in range(2):
                nc.vector.tensor_copy(den[D:D + 1, h01, :],
                                      po[h01][D:D + 1, :])
            for h01 in range(2):
                rb = pp.tile([128, 512], F32, tag="pp")
                nc.tensor.matmul(rb[0:D, :],
                                 ones_t[D:D + 1, :],
                                 den[D:D + 1, h01, :],
                                 start=True, stop=True)
                rec = nrm_pool.tile([128, 512], F32, tag="rec")
                nc.vector.reciprocal_approx_fast(rec[0:D, :], rb[0:D, :])
                nc.vector.tensor_mul(
                    oc[qc][:, 2 * hp + h01, :],
                    po[h01][0:D, :], rec[0:D, :])

        # ---- emission schedule (software pipeline) --------------------
        nc.sync.dma_start(out=wq_s, in_=wq.rearrange("(kt p) c -> p kt c", p=128))
        nc.sync.dma_start(out=wk_s, in_=wk.rearrange("(kt p) c -> p kt c", p=128))
        nc.sync.dma_start(
            out=bq_s, in_=bq.rearrange("(mt p) -> p mt", p=128).unsqueeze(2))
        nc.sync.dma_start(
            out=bk_s, in_=bk.rearrange("(mt p) -> p mt", p=128).unsqueeze(2))
        nc.sync.dma_start(out=wv_s, in_=wv.rearrange("(kt p) c -> p kt c", p=128))
        nc.sync.dma_start(
            out=bv_bc,
            in_=bass.AP(tensor=bv.tensor, offset=bv.offset,
                        ap=[[0, 128]] + list(bv.ap)))
        nc.vector.memset(ones_f[D:D + 1, :], 1.0)
        # touch Exp once so the ACT table loads during the DMA-bound head
        warm = consts.tile([128, 1], F32, tag="warm")
        nc.vector.memset(warm[0:1, :], 0.0)
        nc.scalar.activation(warm[0:1, :], warm[0:1, :], AF.Exp)
        nc.vector.tensor_copy(ones_t[D:D + 1, :], ones_f[D:D + 1, :])
        nc.sync.dma_start(out=wo_s, in_=wo.rearrange("(h p) c -> p h c", p=64))

        emit_a(0)
        emit_b(0)
        g00 = c_iter(0, 0)
        next(g00)                      # tk 0-3 (chunk 0 data)
        for mc in range(1, NMC):
            emit_a(mc)
            emit_b(mc)
            next(g00, None)            # tk blocks as chunks land
        for _ in g00:                  # exhaust (norm)
            pass
        d_queue = []                         # (qc, tl, cc) units, 4 MMs each
        prev_gen = None
        for qc in range(NQC):
            for hp in range(HG // 2):
                if qc == 0 and hp == 0:
                    continue
                gen = c_iter(qc, hp)
                next(gen)                    # prologue: tk 0-3
                if prev_gen is not None:
                    for _ in prev_gen:       # previous iteration's norm
                        pass
                    prev_gen = None
                    if hp == 0 and qc > 0:   # qc-1 fully normalized now
                        d_queue += [(qc - 1, tl, cc)
                                    for tl in range(4) for cc in range(2)]
                if d_queue:
                    emit_d_unit(*d_queue.pop(0))
                for _ in range(3):           # tk 4-15
                    next(gen)
                    if d_queue:
                        emit_d_unit(*d_queue.pop(0))
                prev_gen = gen
        for _ in prev_gen:
            pass
        d_queue += [(NQC - 1, tl, cc) for tl in range(4) for cc in range(2)]
        for u in d_queue:
            emit_d_unit(*u)


_NC_CACHE = None


def _get_program():
    global _NC_CACHE
    if _NC_CACHE is None:
        _NC_CACHE = build_mha_program()
    return _NC_CACHE


def make_in_maps(query, key, value, Wq, bq, Wk, bk, Wv, bv, Wo):
    q = np.asarray(query, np.float32).reshape(B, T, C)
    k = np.asarray(key, np.float32).reshape(B, T, C)
    v = np.asarray(value, np.float32).reshape(B, T, C)
    xT = {n: [np.ascontiguousarray(a[b].T).astype(ml_dtypes.bfloat16)
              for b in range(B)]
          for n, a in (("q", q), ("k", k), ("v", v))}
    in_maps = []
    for c in range(N_CORES):
        b, g = divmod(c, GROUPS)
        sl = slice(g * CG, (g + 1) * CG)
        in_maps.append({
            "xqT": xT["q"][b], "xkT": xT["k"][b], "xvT": xT["v"][b],
            "wq": np.ascontiguousarray(np.asarray(Wq, np.float32)[:, sl]).astype(ml_dtypes.bfloat16),
            "wk": np.ascontiguousarray(np.asarray(Wk, np.float32)[:, sl]).astype(ml_dtypes.bfloat16),
            "wv": np.ascontiguousarray(np.asarray(Wv, np.float32)[:, sl]).astype(ml_dtypes.bfloat16),
            "bq": np.ascontiguousarray(np.asarray(bq, np.float32)[sl]),
            "bk": np.ascontiguousarray(np.asarray(bk, np.float32)[sl]),
            "bv": np.ascontiguousarray(np.asarray(bv, np.float32)[sl]),
            "wo": np.ascontiguousarray(np.asarray(Wo, np.float32)[sl, :]).astype(ml_dtypes.bfloat16),
        })
    return in_maps


def assemble_output(results, bo):
    y = np.zeros((B, T, C), np.float32)
    for c, res in enumerate(results):
        y[c // GROUPS] += res["yp"]
    y += np.asarray(bo, np.float32)
    return y


def kernel(query, key, value, Wq, bq, Wk, bk, Wv, bv, Wo, bo):
    nc = _get_program()
    in_maps = make_in_maps(query, key, value, Wq, bq, Wk, bk, Wv, bv, Wo)
    res = run_bass_kernel_spmd(nc, in_maps, list(range(N_CORES)))
    return assemble_output(res.results, bo)
